# revision 10
# baseline (speedup 1.0000x reference)
"""Causal self-attention (GPT-style block) on 8 Trainium2 NeuronCores.

Sharding: pure data-parallel over batch. B=8 batch elements map 1:1 onto the
8 cores; every core runs the full per-sequence attention, so no collectives
are needed and the load is perfectly balanced.

Per-core math (T=1024, C=768, H=12, hd=64), all derived layouts keep the
matmul contraction dim on SBUF partitions:
  1. x [T,C] and w_attn/w_proj are transposed on-chip via the PE array.
  2. qkv^T = w_attn @ x^T in fp32r (full-rate fp32 mode): q^T,k^T land as
     [o,t] chunks (a head PAIR per 128-partition chunk); v lands natural
     [t,o] augmented with a ones column per head for fused softmax sums.
  3. Per head: S^T = k @ q^T (both heads of a chunk run concurrently via
     PE row-tiling, K=64 each). exp() on ScalarE with the 1/sqrt(hd) scale
     folded in; no max-subtraction (scores are O(1) for this problem's
     distribution, fp32 exp cannot overflow). Causality by skipping the
     fully-masked chunk pairs plus one triangular mask-multiply on the
     diagonal 128x128 block.
  4. y = P @ v computed with expS^T slices as the stationary operand in
     bf16: out[tq, 64+1] accumulates over tk chunks; column 64 is the
     softmax denominator (from the ones column). Normalization is then one
     per-partition reciprocal + tensor_scalar multiply.
  5. y is PE-transposed to y^T and projected: out = y @ w_proj^T in fp32r.
"""

import os
import sys
from contextlib import ExitStack

import numpy as np

if "/opt/trn_rl_repo" not in sys.path:
    sys.path.insert(0, "/opt/trn_rl_repo")

import concourse.bacc as bacc
import concourse.bass as bass
import concourse.tile as tile
from concourse import mybir
from concourse.masks import make_identity, make_upper_triangular

F32 = mybir.dt.float32
F32R = mybir.dt.float32r
BF16 = mybir.dt.bfloat16

T = 1024
C = 768
H = 12
HD = C // H  # 64
N_CORES = 8


def ceil_div(a, b):
    return (a + b - 1) // b


def build_attention_core(t=T):
    """Build the single-core Bass program (SPMD across 8 cores)."""
    nc = bacc.Bacc(None, target_bir_lowering=False, debug=False)
    x = nc.declare_dram_parameter("x", [t, C], F32, isOutput=False)
    w_attn = nc.declare_dram_parameter("w_attn", [3 * C, C], F32, isOutput=False)
    b_attn = nc.declare_dram_parameter("b_attn", [3 * C], F32, isOutput=False)
    w_proj = nc.declare_dram_parameter("w_proj", [C, C], F32, isOutput=False)
    b_proj = nc.declare_dram_parameter("b_proj", [C], F32, isOutput=False)
    out = nc.declare_dram_parameter("out", [t, C], F32, isOutput=True)

    NT = t // 128  # t-chunks
    NCC = C // 128  # c-chunks (6)
    NHP = H // 2  # head pairs (6)

    with ExitStack() as ctx:
        tc = ctx.enter_context(tile.TileContext(nc))
        singles = ctx.enter_context(tc.tile_pool(name="singles", bufs=1))
        psum = ctx.enter_context(tc.tile_pool(name="psum", bufs=1, space="PSUM"))

        # ---- constants -------------------------------------------------
        ident = singles.tile([128, 128], F32)
        make_identity(nc, ident)
        # keep-mask for the diagonal S^T block: 1.0 where tk(part) <= tq(col)
        tri = singles.tile([128, 128], BF16)
        make_upper_triangular(nc, tri, val=1.0, diag=True)

        # b_attn[0:2*C] rearranged so column j holds the per-partition bias
        # of qk o-chunk j ([128,1] slices for tensor_scalar_add).
        bias_qk = singles.tile([128, 2 * NCC], F32)
        nc.sync.dma_start(
            out=bias_qk,
            in_=b_attn[0 : 2 * C].rearrange("(c p) -> p c", p=128),
        )
        # v bias broadcast along partitions: [128, C]
        bias_v = singles.tile([128, C], F32)
        bav = b_attn[2 * C : 3 * C].rearrange("(o c) -> o c", o=1)
        nc.gpsimd.dma_start(
            out=bias_v,
            in_=bass.AP(tensor=bav.tensor, offset=bav.offset, ap=[[0, 128]] + bav.ap[1:]),
        )
        bias_p = singles.tile([128, C], F32)
        bpv = b_proj[:].rearrange("(o c) -> o c", o=1)
        nc.gpsimd.dma_start(
            out=bias_p,
            in_=bass.AP(tensor=bpv.tensor, offset=bpv.offset, ap=[[0, 128]] + bpv.ap[1:]),
        )

        def n_pieces(total, maxw=512):
            """split [0,total) into <=maxw pieces"""
            res = []
            s = 0
            while s < total:
                w = min(maxw, total - s)
                res.append((s, w))
                s += w
            return res

        # Pools are stack-allocated in entry order and close LIFO, so nest
        # them by actual tensor lifetime to keep the peak footprint low:
        #   pool_y  (y_nat):          phases C..D
        #   pool_qkv (qT/kT/v_aug):   phases B..C   (closes before D)
        #     pool1 (x/w transposes): phases A..B
        #     pool_att (expS, rcp):   phase C
        #   pool_de (yT/wpT/out):     phases D..E   (reuses pool_qkv space)
        pool_y = ctx.enter_context(tc.tile_pool(name="pool_y", bufs=1))
        y_nat = [pool_y.tile([128, C], F32, name=f"ynat{j}") for j in range(NT)]

        pool2_cm = tc.tile_pool(name="pool_qkv", bufs=1)
        pool2 = pool2_cm.__enter__()

        qT = [pool2.tile([128, t], F32R, name=f"qT{j}") for j in range(NHP)]
        kT = [pool2.tile([128, t], F32R, name=f"kT{j}") for j in range(NHP)]
        # v augmented with a ones column per head: [128, H, HD+1] per t-chunk
        v_aug = [pool2.tile([128, H, HD + 1], BF16, name=f"vaug{i}") for i in range(NT)]

        # ================= phase A+B: transposes + qkv =================
        with tc.tile_pool(name="pool1", bufs=1) as pool1:
            # x natural -> xT chunks [c-part, t-free]
            xT = [pool1.tile([128, t], F32R, name=f"xT{cc}") for cc in range(NCC)]
            for it in range(NT):
                xn = pool1.tile([128, C], F32, name="xn", bufs=3)
                nc.sync.dma_start(out=xn, in_=x[it * 128 : (it + 1) * 128, :])
                for cc in range(NCC):
                    ps = psum.tile([128, 128], F32, name="ps_tr", tag="ps_tr", bufs=2)
                    nc.tensor.transpose(ps, xn[:, cc * 128 : (cc + 1) * 128], ident)
                    nc.vector.tensor_copy(xT[cc][:, it * 128 : (it + 1) * 128], ps)

            for i in range(NT):
                nc.vector.memset(v_aug[i][:, :, HD : HD + 1], 1.0)

            # qkv projection, streaming w_attn o-chunks
            for og in range(3 * NCC):
                wn = pool1.tile([128, C], F32, name="wn", bufs=3)
                nc.sync.dma_start(out=wn, in_=w_attn[og * 128 : (og + 1) * 128, :])
                # transpose this o-slab: waTs[cc] = w_attn[og-block, cc-block]^T
                waTs = []
                for cc in range(NCC):
                    ps = psum.tile([128, 128], F32, name="ps_tr2", tag="ps_tr", bufs=2)
                    nc.tensor.transpose(ps, wn[:, cc * 128 : (cc + 1) * 128], ident)
                    wt = pool1.tile([128, 128], F32R, name=f"waTs{cc}", tag=f"waTs{cc}", bufs=2)
                    nc.vector.tensor_copy(wt, ps)
                    waTs.append(wt)

                if og < 2 * NCC:
                    # q^T / k^T orientation: out[o-part, t-free]
                    for (s, w) in n_pieces(t):
                        pq = psum.tile([128, 512], F32, name="ps_mm", tag="ps_mm", bufs=3)
                        for cc in range(NCC):
                            nc.tensor.matmul(
                                pq[:, :w],
                                waTs[cc],
                                xT[cc][:, s : s + w],
                                start=(cc == 0),
                                stop=(cc == NCC - 1),
                            )
                        dst = qT[og] if og < NCC else kT[og - NCC]
                        nc.vector.tensor_scalar_add(
                            dst[:, s : s + w], pq[:, :w], bias_qk[:, og : og + 1]
                        )
                else:
                    # v orientation: out[t-part, o-free]; og covers heads
                    # [2*(og-12), 2*(og-12)+2), i.e. o-cols [128*(og-12), +128)
                    vg = og - 2 * NCC
                    for it in range(NT):
                        pv = psum.tile([128, 128], F32, name="ps_v", tag="ps_tr", bufs=2)
                        for cc in range(NCC):
                            nc.tensor.matmul(
                                pv,
                                xT[cc][:, it * 128 : (it + 1) * 128],
                                waTs[cc],
                                start=(cc == 0),
                                stop=(cc == NCC - 1),
                            )
                        nc.vector.tensor_add(
                            v_aug[it][:, 2 * vg : 2 * vg + 2, 0:HD],
                            pv.rearrange("p (h d) -> p h d", d=HD),
                            bias_v[:, 128 * vg : 128 * (vg + 1)].rearrange(
                                "p (h d) -> p h d", d=HD
                            ),
                        )

        # ================= phase C: attention ==========================
        pool3_cm = tc.tile_pool(name="pool_att", bufs=1)
        pool3 = pool3_cm.__enter__()
        for hp in range(NHP):
            hA, hB = 2 * hp, 2 * hp + 1
            # expS^T[i] tiles for both heads of the pair, bf16
            eA = [
                pool3.tile([128, t], BF16, name=f"eA{i}", tag=f"eA{i}", bufs=2)
                for i in range(NT)
            ]
            eB = [
                pool3.tile([128, t], BF16, name=f"eB{i}", tag=f"eB{i}", bufs=2)
                for i in range(NT)
            ]
            for i in range(NT):
                # S^T chunk: out[tk 128i.., tq 128i..t); both heads concurrent
                # via PE row-tiling (K=64 at partitions 0-63 / 64-127).
                for (s, w) in n_pieces(t - 128 * i):
                    tq0 = 128 * i + s
                    for head, half, e in ((hA, 0, eA), (hB, 64, eB)):
                        ps = psum.tile(
                            [128, 512], F32, name="ps_s", tag="ps_mm", bufs=3
                        )
                        nc.tensor.matmul(
                            ps[:, :w],
                            kT[hp][half : half + 64, 128 * i : 128 * (i + 1)],
                            qT[hp][half : half + 64, tq0 : tq0 + w],
                            start=True,
                            stop=True,
                        )
                        nc.scalar.activation(
                            e[i][:, tq0 : tq0 + w],
                            ps[:, :w],
                            mybir.ActivationFunctionType.Exp,
                            bias=0.0,
                            scale=1.0 / float(np.sqrt(HD)),
                        )
                # causal mask on the diagonal block (keep tk <= tq)
                d0 = 128 * i
                nc.vector.tensor_mul(eA[i][:, d0 : d0 + 128], eA[i][:, d0 : d0 + 128], tri)
                nc.vector.tensor_mul(eB[i][:, d0 : d0 + 128], eB[i][:, d0 : d0 + 128], tri)

            # PV: for each tq chunk j accumulate over tk chunks i<=j.
            for head, e in ((hA, eA), (hB, eB)):
                for j in range(NT):
                    py = psum.tile([128, HD + 1], F32, name="ps_y", tag="ps_y", bufs=2)
                    for i in range(j + 1):
                        nc.tensor.matmul(
                            py,
                            e[i][:, 128 * j : 128 * (j + 1)],
                            v_aug[i][:, head, :],
                            start=(i == 0),
                            stop=(i == j),
                        )
                    rcp = pool3.tile([128, 1], F32, name="rcp", tag="rcp", bufs=4)
                    nc.vector.reciprocal(rcp, py[:, HD : HD + 1])
                    nc.vector.tensor_scalar_mul(
                        y_nat[j][:, head * HD : (head + 1) * HD], py[:, 0:HD], rcp
                    )

        pool3_cm.__exit__(None, None, None)
        pool2_cm.__exit__(None, None, None)

        # ================= phase D+E: transpose y, project =============
        pool4 = ctx.enter_context(tc.tile_pool(name="pool_de", bufs=1))
        yT = [pool4.tile([128, t], F32R, name=f"yT{cc}") for cc in range(NCC)]
        for j in range(NT):
            for cc in range(NCC):
                ps = psum.tile([128, 128], F32, name="ps_try", tag="ps_tr", bufs=2)
                nc.tensor.transpose(ps, y_nat[j][:, cc * 128 : (cc + 1) * 128], ident)
                nc.vector.tensor_copy(yT[cc][:, j * 128 : (j + 1) * 128], ps)

        wpT = [pool4.tile([128, C], F32R, name=f"wpT{cc}") for cc in range(NCC)]
        for og in range(NCC):
            wpn = pool4.tile([128, C], F32, name="wpn", bufs=2)
            nc.sync.dma_start(out=wpn, in_=w_proj[og * 128 : (og + 1) * 128, :])
            for cc in range(NCC):
                ps = psum.tile([128, 128], F32, name="ps_trp", tag="ps_tr", bufs=2)
                nc.tensor.transpose(ps, wpn[:, cc * 128 : (cc + 1) * 128], ident)
                nc.vector.tensor_copy(wpT[cc][:, og * 128 : (og + 1) * 128], ps)

        for it in range(NT):
            out_sb = pool4.tile([128, C], F32, name="out_sb", bufs=3)
            for (s, w) in n_pieces(C):
                po = psum.tile([128, 512], F32, name="ps_o", tag="ps_mm", bufs=3)
                for cc in range(NCC):
                    nc.tensor.matmul(
                        po[:, :w],
                        yT[cc][:, it * 128 : (it + 1) * 128],
                        wpT[cc][:, s : s + w],
                        start=(cc == 0),
                        stop=(cc == NCC - 1),
                    )
                nc.vector.tensor_add(
                    out_sb[:, s : s + w], po[:, :w], bias_p[:, s : s + w]
                )
            nc.sync.dma_start(out=out[it * 128 : (it + 1) * 128, :], in_=out_sb)

    nc.compile()
    return nc


_NC_CACHE = {}


def get_nc(t=T):
    if t not in _NC_CACHE:
        _NC_CACHE[t] = build_attention_core(t)
    return _NC_CACHE[t]


def kernel(**inputs):
    from concourse.bass_utils import run_bass_kernel_spmd

    x = np.ascontiguousarray(inputs["x"], dtype=np.float32)
    w_attn = np.ascontiguousarray(inputs["w_attn"], dtype=np.float32)
    b_attn = np.ascontiguousarray(inputs["b_attn"], dtype=np.float32)
    w_proj = np.ascontiguousarray(inputs["w_proj"], dtype=np.float32)
    b_proj = np.ascontiguousarray(inputs["b_proj"], dtype=np.float32)
    B = x.shape[0]
    assert B == N_CORES

    nc = get_nc(x.shape[1])
    in_maps = [
        {
            "x": x[b],
            "w_attn": w_attn,
            "b_attn": b_attn,
            "w_proj": w_proj,
            "b_proj": b_proj,
        }
        for b in range(B)
    ]
    res = run_bass_kernel_spmd(nc, in_maps, core_ids=list(range(N_CORES)))
    return np.stack([res.results[b]["out"] for b in range(B)]).astype(np.float32)


# revision 12
# speedup vs baseline: 8.0890x; 8.0890x over previous
"""Causal self-attention (GPT-style block) on 8 Trainium2 NeuronCores.

Sharding: pure data-parallel over batch. B=8 batch elements map 1:1 onto the
8 cores; every core runs the full per-sequence attention, so no collectives
are needed and the load is perfectly balanced.

Per-core math (T=1024, C=768, H=12, hd=64), all derived layouts keep the
matmul contraction dim on SBUF partitions:
  1. x [T,C] and w_attn/w_proj are transposed on-chip via the PE array.
  2. qkv^T = w_attn @ x^T in fp32r (full-rate fp32 mode): q^T,k^T land as
     [o,t] chunks (a head PAIR per 128-partition chunk); v lands natural
     [t,o] augmented with a ones column per head for fused softmax sums.
  3. Per head: S^T = k @ q^T (both heads of a chunk run concurrently via
     PE row-tiling, K=64 each). exp() on ScalarE with the 1/sqrt(hd) scale
     folded in; no max-subtraction (scores are O(1) for this problem's
     distribution, fp32 exp cannot overflow). Causality by skipping the
     fully-masked chunk pairs plus one triangular mask-multiply on the
     diagonal 128x128 block.
  4. y = P @ v computed with expS^T slices as the stationary operand in
     bf16: out[tq, 64+1] accumulates over tk chunks; column 64 is the
     softmax denominator (from the ones column). Normalization is then one
     per-partition reciprocal + tensor_scalar multiply.
  5. y is PE-transposed to y^T and projected: out = y @ w_proj^T in fp32r.
"""

import os
import sys
from contextlib import ExitStack

import numpy as np

if "/opt/trn_rl_repo" not in sys.path:
    sys.path.insert(0, "/opt/trn_rl_repo")

import concourse.bacc as bacc
import concourse.bass as bass
import concourse.tile as tile
from concourse import mybir
from concourse.masks import make_identity, make_upper_triangular

F32 = mybir.dt.float32
F32R = mybir.dt.float32r
BF16 = mybir.dt.bfloat16

T = 1024
C = 768
H = 12
HD = C // H  # 64
N_CORES = 8


def ceil_div(a, b):
    return (a + b - 1) // b


def build_attention_core(t=T, repeats=1):
    """Build the single-core Bass program (SPMD across 8 cores).

    repeats>1 emits the whole computation that many times into one NEFF —
    used only for benchmarking (amortizes host dispatch overhead).
    """
    nc = bacc.Bacc(None, target_bir_lowering=False, debug=False)
    x = nc.declare_dram_parameter("x", [t, C], F32, isOutput=False)
    w_attn = nc.declare_dram_parameter("w_attn", [3 * C, C], F32, isOutput=False)
    b_attn = nc.declare_dram_parameter("b_attn", [3 * C], F32, isOutput=False)
    w_proj = nc.declare_dram_parameter("w_proj", [C, C], F32, isOutput=False)
    b_proj = nc.declare_dram_parameter("b_proj", [C], F32, isOutput=False)
    out = nc.declare_dram_parameter("out", [t, C], F32, isOutput=True)

    with ExitStack() as octx:
        tc = octx.enter_context(tile.TileContext(nc))
        for _rep in range(repeats):
            _emit_once(nc, tc, t, x, w_attn, b_attn, w_proj, b_proj, out)
    nc.compile()
    return nc


def _emit_once(nc, tc, t, x, w_attn, b_attn, w_proj, b_proj, out):
    NT = t // 128  # t-chunks
    NCC = C // 128  # c-chunks (6)
    NHP = H // 2  # head pairs (6)

    with ExitStack() as ctx:
        singles = ctx.enter_context(tc.tile_pool(name="singles", bufs=1))
        psum = ctx.enter_context(tc.tile_pool(name="psum", bufs=1, space="PSUM"))

        # ---- constants -------------------------------------------------
        ident = singles.tile([128, 128], F32)
        make_identity(nc, ident)
        # keep-mask for the diagonal S^T block: 1.0 where tk(part) <= tq(col)
        tri = singles.tile([128, 128], BF16)
        make_upper_triangular(nc, tri, val=1.0, diag=True)

        # b_attn[0:2*C] rearranged so column j holds the per-partition bias
        # of qk o-chunk j ([128,1] slices for tensor_scalar_add).
        bias_qk = singles.tile([128, 2 * NCC], F32)
        nc.sync.dma_start(
            out=bias_qk,
            in_=b_attn[0 : 2 * C].rearrange("(c p) -> p c", p=128),
        )
        # v bias broadcast along partitions: [128, C]
        bias_v = singles.tile([128, C], F32)
        bav = b_attn[2 * C : 3 * C].rearrange("(o c) -> o c", o=1)
        nc.gpsimd.dma_start(
            out=bias_v,
            in_=bass.AP(tensor=bav.tensor, offset=bav.offset, ap=[[0, 128]] + bav.ap[1:]),
        )
        bias_p = singles.tile([128, C], F32)
        bpv = b_proj[:].rearrange("(o c) -> o c", o=1)
        nc.gpsimd.dma_start(
            out=bias_p,
            in_=bass.AP(tensor=bpv.tensor, offset=bpv.offset, ap=[[0, 128]] + bpv.ap[1:]),
        )

        def n_pieces(total, maxw=512):
            """split [0,total) into <=maxw pieces"""
            res = []
            s = 0
            while s < total:
                w = min(maxw, total - s)
                res.append((s, w))
                s += w
            return res

        # Pools are stack-allocated in entry order and close LIFO, so nest
        # them by actual tensor lifetime to keep the peak footprint low:
        #   pool_y  (y_nat):          phases C..D
        #   pool_qkv (qT/kT/v_aug):   phases B..C   (closes before D)
        #     pool1 (x/w transposes): phases A..B
        #     pool_att (expS, rcp):   phase C
        #   pool_de (yT/wpT/out):     phases D..E   (reuses pool_qkv space)
        pool_y = ctx.enter_context(tc.tile_pool(name="pool_y", bufs=1))
        y_nat = [pool_y.tile([128, C], F32, name=f"ynat{j}") for j in range(NT)]

        pool2_cm = tc.tile_pool(name="pool_qkv", bufs=1)
        pool2 = pool2_cm.__enter__()

        qT = [pool2.tile([128, t], F32R, name=f"qT{j}") for j in range(NHP)]
        kT = [pool2.tile([128, t], F32R, name=f"kT{j}") for j in range(NHP)]
        # v augmented with a ones column per head: [128, H, HD+1] per t-chunk
        v_aug = [pool2.tile([128, H, HD + 1], BF16, name=f"vaug{i}") for i in range(NT)]

        # ================= phase A+B: transposes + qkv =================
        with tc.tile_pool(name="pool1", bufs=1) as pool1:
            # x natural -> xT chunks [c-part, t-free]
            xT = [pool1.tile([128, t], F32R, name=f"xT{cc}") for cc in range(NCC)]
            for it in range(NT):
                xn = pool1.tile([128, C], F32, name="xn", bufs=3)
                nc.sync.dma_start(out=xn, in_=x[it * 128 : (it + 1) * 128, :])
                for cc in range(NCC):
                    ps = psum.tile([128, 128], F32, name="ps_tr", tag="ps_tr", bufs=2)
                    nc.tensor.transpose(ps, xn[:, cc * 128 : (cc + 1) * 128], ident)
                    nc.vector.tensor_copy(xT[cc][:, it * 128 : (it + 1) * 128], ps)

            for i in range(NT):
                nc.vector.memset(v_aug[i][:, :, HD : HD + 1], 1.0)

            # qkv projection, streaming w_attn o-chunks
            for og in range(3 * NCC):
                wn = pool1.tile([128, C], F32, name="wn", bufs=3)
                nc.sync.dma_start(out=wn, in_=w_attn[og * 128 : (og + 1) * 128, :])
                # transpose this o-slab: waTs[cc] = w_attn[og-block, cc-block]^T
                waTs = []
                for cc in range(NCC):
                    ps = psum.tile([128, 128], F32, name="ps_tr2", tag="ps_tr", bufs=2)
                    nc.tensor.transpose(ps, wn[:, cc * 128 : (cc + 1) * 128], ident)
                    wt = pool1.tile([128, 128], F32R, name=f"waTs{cc}", tag=f"waTs{cc}", bufs=2)
                    nc.vector.tensor_copy(wt, ps)
                    waTs.append(wt)

                if og < 2 * NCC:
                    # q^T / k^T orientation: out[o-part, t-free]
                    for (s, w) in n_pieces(t):
                        pq = psum.tile([128, 512], F32, name="ps_mm", tag="ps_mm", bufs=3)
                        for cc in range(NCC):
                            nc.tensor.matmul(
                                pq[:, :w],
                                waTs[cc],
                                xT[cc][:, s : s + w],
                                start=(cc == 0),
                                stop=(cc == NCC - 1),
                            )
                        dst = qT[og] if og < NCC else kT[og - NCC]
                        nc.vector.tensor_scalar_add(
                            dst[:, s : s + w], pq[:, :w], bias_qk[:, og : og + 1]
                        )
                else:
                    # v orientation: out[t-part, o-free]; og covers heads
                    # [2*(og-12), 2*(og-12)+2), i.e. o-cols [128*(og-12), +128)
                    vg = og - 2 * NCC
                    for it in range(NT):
                        pv = psum.tile([128, 128], F32, name="ps_v", tag="ps_tr", bufs=2)
                        for cc in range(NCC):
                            nc.tensor.matmul(
                                pv,
                                xT[cc][:, it * 128 : (it + 1) * 128],
                                waTs[cc],
                                start=(cc == 0),
                                stop=(cc == NCC - 1),
                            )
                        nc.vector.tensor_add(
                            v_aug[it][:, 2 * vg : 2 * vg + 2, 0:HD],
                            pv.rearrange("p (h d) -> p h d", d=HD),
                            bias_v[:, 128 * vg : 128 * (vg + 1)].rearrange(
                                "p (h d) -> p h d", d=HD
                            ),
                        )

        # ================= phase C: attention ==========================
        pool3_cm = tc.tile_pool(name="pool_att", bufs=1)
        pool3 = pool3_cm.__enter__()
        for hp in range(NHP):
            hA, hB = 2 * hp, 2 * hp + 1
            # expS^T[i] tiles for both heads of the pair, bf16
            eA = [
                pool3.tile([128, t], BF16, name=f"eA{i}", tag=f"eA{i}", bufs=2)
                for i in range(NT)
            ]
            eB = [
                pool3.tile([128, t], BF16, name=f"eB{i}", tag=f"eB{i}", bufs=2)
                for i in range(NT)
            ]
            for i in range(NT):
                # S^T chunk: out[tk 128i.., tq 128i..t); both heads concurrent
                # via PE row-tiling (K=64 at partitions 0-63 / 64-127).
                for (s, w) in n_pieces(t - 128 * i):
                    tq0 = 128 * i + s
                    for head, half, e in ((hA, 0, eA), (hB, 64, eB)):
                        ps = psum.tile(
                            [128, 512], F32, name="ps_s", tag="ps_mm", bufs=3
                        )
                        nc.tensor.matmul(
                            ps[:, :w],
                            kT[hp][half : half + 64, 128 * i : 128 * (i + 1)],
                            qT[hp][half : half + 64, tq0 : tq0 + w],
                            start=True,
                            stop=True,
                        )
                        nc.scalar.activation(
                            e[i][:, tq0 : tq0 + w],
                            ps[:, :w],
                            mybir.ActivationFunctionType.Exp,
                            bias=0.0,
                            scale=1.0 / float(np.sqrt(HD)),
                        )
                # causal mask on the diagonal block (keep tk <= tq)
                d0 = 128 * i
                nc.vector.tensor_mul(eA[i][:, d0 : d0 + 128], eA[i][:, d0 : d0 + 128], tri)
                nc.vector.tensor_mul(eB[i][:, d0 : d0 + 128], eB[i][:, d0 : d0 + 128], tri)

            # PV: for each tq chunk j accumulate over tk chunks i<=j.
            for head, e in ((hA, eA), (hB, eB)):
                for j in range(NT):
                    py = psum.tile([128, HD + 1], F32, name="ps_y", tag="ps_y", bufs=2)
                    for i in range(j + 1):
                        nc.tensor.matmul(
                            py,
                            e[i][:, 128 * j : 128 * (j + 1)],
                            v_aug[i][:, head, :],
                            start=(i == 0),
                            stop=(i == j),
                        )
                    rcp = pool3.tile([128, 1], F32, name="rcp", tag="rcp", bufs=4)
                    nc.vector.reciprocal(rcp, py[:, HD : HD + 1])
                    nc.vector.tensor_scalar_mul(
                        y_nat[j][:, head * HD : (head + 1) * HD], py[:, 0:HD], rcp
                    )

        pool3_cm.__exit__(None, None, None)
        pool2_cm.__exit__(None, None, None)

        # ================= phase D+E: transpose y, project =============
        pool4 = ctx.enter_context(tc.tile_pool(name="pool_de", bufs=1))
        yT = [pool4.tile([128, t], F32R, name=f"yT{cc}") for cc in range(NCC)]
        for j in range(NT):
            for cc in range(NCC):
                ps = psum.tile([128, 128], F32, name="ps_try", tag="ps_tr", bufs=2)
                nc.tensor.transpose(ps, y_nat[j][:, cc * 128 : (cc + 1) * 128], ident)
                nc.vector.tensor_copy(yT[cc][:, j * 128 : (j + 1) * 128], ps)

        wpT = [pool4.tile([128, C], F32R, name=f"wpT{cc}") for cc in range(NCC)]
        for og in range(NCC):
            wpn = pool4.tile([128, C], F32, name="wpn", bufs=2)
            nc.sync.dma_start(out=wpn, in_=w_proj[og * 128 : (og + 1) * 128, :])
            for cc in range(NCC):
                ps = psum.tile([128, 128], F32, name="ps_trp", tag="ps_tr", bufs=2)
                nc.tensor.transpose(ps, wpn[:, cc * 128 : (cc + 1) * 128], ident)
                nc.vector.tensor_copy(wpT[cc][:, og * 128 : (og + 1) * 128], ps)

        for it in range(NT):
            out_sb = pool4.tile([128, C], F32, name="out_sb", bufs=3)
            for (s, w) in n_pieces(C):
                po = psum.tile([128, 512], F32, name="ps_o", tag="ps_mm", bufs=3)
                for cc in range(NCC):
                    nc.tensor.matmul(
                        po[:, :w],
                        yT[cc][:, it * 128 : (it + 1) * 128],
                        wpT[cc][:, s : s + w],
                        start=(cc == 0),
                        stop=(cc == NCC - 1),
                    )
                nc.vector.tensor_add(
                    out_sb[:, s : s + w], po[:, :w], bias_p[:, s : s + w]
                )
            nc.sync.dma_start(out=out[it * 128 : (it + 1) * 128, :], in_=out_sb)


_NC_CACHE = {}


def get_nc(t=T):
    if t not in _NC_CACHE:
        _NC_CACHE[t] = build_attention_core(t)
    return _NC_CACHE[t]


def kernel(**inputs):
    from concourse.bass_utils import run_bass_kernel_spmd

    x = np.ascontiguousarray(inputs["x"], dtype=np.float32)
    w_attn = np.ascontiguousarray(inputs["w_attn"], dtype=np.float32)
    b_attn = np.ascontiguousarray(inputs["b_attn"], dtype=np.float32)
    w_proj = np.ascontiguousarray(inputs["w_proj"], dtype=np.float32)
    b_proj = np.ascontiguousarray(inputs["b_proj"], dtype=np.float32)
    B = x.shape[0]
    assert B == N_CORES

    nc = get_nc(x.shape[1])
    in_maps = [
        {
            "x": x[b],
            "w_attn": w_attn,
            "b_attn": b_attn,
            "w_proj": w_proj,
            "b_proj": b_proj,
        }
        for b in range(B)
    ]
    res = run_bass_kernel_spmd(nc, in_maps, core_ids=list(range(N_CORES)))
    return np.stack([res.results[b]["out"] for b in range(B)]).astype(np.float32)
